# revision 32
# baseline (speedup 1.0000x reference)
"""Trainium2 Bass kernel for pointer-generator coverage attention.

reference math (per batch b):
    enc_feat = enc @ We.T                      # [T, H]
    dec      = s_t_hat @ Wd.T + bd             # [H]      (computed on host: 0.003% of FLOPs)
    feat     = enc_feat + dec + coverage[:,None]*wc
    e        = tanh(feat)
    scores   = e @ v                           # [T]
    p        = exp(scores) * mask              # (softmax max-subtraction skipped:
    attn     = p / sum(p)                      #  |scores| < ~1, exp is safe in fp32)
    c_t      = attn @ enc                      # [H]
    new_cov  = clip(coverage + attn, 0, 1)

Distribution: data-parallel over batch, 4 batches per NeuronCore x 8 cores.
Weights replicated. No collectives; gather on host.

On-chip layout: feat^T tiles [g=128, t] so that
  - feat matmul:  lhsT = We^T chunk [h,g] (stationary), rhs = enc^T [h,t] (moving)
  - cov/dec fold: extra K=2 matmul, lhsT = [wc; dec] [2,g], rhs = [cov; 1] [2,t]
  - scores:       lhsT = tanh tile [g, t128] (stationary), rhs = v [g,1] -> out [t128, 1]
    which lands scores partition-major: softmax is [128, 64] per batch.
enc is shipped twice from host (bf16): transposed [h, t] for the feat matmul and
tile-natural [t, h] for the c_t matmul (PE contracts over the partition dim, and
on-chip bulk transposition is more expensive than the extra DMA).
"""

import contextlib

import numpy as np
import ml_dtypes

import concourse.bass as bass
import concourse.tile as tile
from concourse import bacc
from concourse import mybir
from concourse.bass_utils import run_bass_kernel_spmd
from concourse.masks import make_identity

B, T, H = 32, 8192, 256
NCORES = 8
BL = B // NCORES          # 4 batches per core
HC = H // 128             # 2 contraction chunks
NT128 = T // 128          # 64 tiles of 128 tokens
TBLK = 1024               # t-span per feat psum tile (2 PSUM banks)
NBLK = T // TBLK          # 8
MMN = 512                 # max matmul free dim (one PSUM bank)

BF16 = mybir.dt.bfloat16
F32 = mybir.dt.float32
AF = mybir.ActivationFunctionType
OP = mybir.AluOpType

_nc_cache = []
LAST = {}  # stash of the most recent BassKernelResults (for profiling in test.py)


def _build_program(stage=5, reps=1):
    """stage: 1=DMA only, 2=+feat/tanh, 3=+scores, 4=+softmax, 5=full.
    reps>1 wraps the body in a hardware loop (timing builds only)."""
    nc = bacc.Bacc(None)

    encT = nc.dram_tensor("encT", [BL, HC, 128, T], BF16, kind="ExternalInput")
    encN = nc.dram_tensor("encN", [BL, 128, NT128, H], BF16, kind="ExternalInput")
    weT = nc.dram_tensor("weT", [HC, 128, H], BF16, kind="ExternalInput")
    wcdec = nc.dram_tensor("wcdec", [BL, 2, H], BF16, kind="ExternalInput")
    vt = nc.dram_tensor("vt", [H, 1], BF16, kind="ExternalInput")
    covb = nc.dram_tensor("covb", [BL, 2, T], BF16, kind="ExternalInput")
    covf = nc.dram_tensor("covf", [BL, NT128, 128], F32, kind="ExternalInput")
    maskT = nc.dram_tensor("maskT", [BL, 128, NT128], F32, kind="ExternalInput")

    attn_out = nc.dram_tensor("attn_out", [BL, NT128, 128], F32, kind="ExternalOutput")
    ncov_out = nc.dram_tensor("ncov_out", [BL, NT128, 128], F32, kind="ExternalOutput")
    ct_out = nc.dram_tensor("ct_out", [BL, H], F32, kind="ExternalOutput")

    with tile.TileContext(nc) as tc:
        with (
            tc.tile_pool(name="const", bufs=1) as const,
            tc.tile_pool(name="enc", bufs=2) as encpool,
            tc.tile_pool(name="th", bufs=3) as thpool,
            tc.tile_pool(name="sm", bufs=2) as smpool,
            tc.tile_pool(name="outp", bufs=2) as outpool,
            tc.tile_pool(name="fps", bufs=2, space="PSUM") as fpsum,
            tc.tile_pool(name="sps", bufs=1, space="PSUM") as spsum,
            tc.tile_pool(name="tiny", bufs=2, space="PSUM") as tinypsum,
        ):
            # ---- constants, loaded once ----
            weT_sb = const.tile([128, HC, H], BF16)
            for hc in range(HC):
                nc.sync.dma_start(weT_sb[:, hc, :], weT[hc])
            vt_sb = const.tile([128, HC], BF16)
            for gc in range(HC):
                nc.sync.dma_start(vt_sb[:, gc : gc + 1], vt[gc * 128 : (gc + 1) * 128])
            wcdec_sb = const.tile([2, BL, H], BF16)
            for b in range(BL):
                nc.sync.dma_start(wcdec_sb[:, b, :], wcdec[b])
            ones128 = const.tile([128, 1], F32)
            nc.vector.memset(ones128[:], 1.0)
            onesrow = const.tile([1, 128], F32)
            nc.vector.memset(onesrow[:], 1.0)
            ident = const.tile([128, 128], F32)
            make_identity(nc, ident[:])

            _loop = contextlib.ExitStack()
            if reps > 1:
                _loop.enter_context(tc.For_i(0, reps, 1))
            for b in range(BL):
                # ---- per-batch streamed inputs ----
                encT_sb = encpool.tile([128, HC, T], BF16, tag="encT")
                for hc in range(HC):
                    nc.sync.dma_start(encT_sb[:, hc, :], encT[b, hc])
                encN_sb = encpool.tile([128, NT128, H], BF16, tag="encN")
                nc.sync.dma_start(encN_sb[:], encN[b])
                covone = encpool.tile([2, T], BF16, tag="covone")
                nc.sync.dma_start(covone[:], covb[b])
                maskT_sb = smpool.tile([128, NT128], F32, tag="maskT")
                nc.sync.dma_start(maskT_sb[:], maskT[b])

                if stage >= 3:
                    s_ps = spsum.tile([128, NT128], F32, tag="s")
                else:
                    s_ps = None

                if stage < 2:
                    ncov_rows = outpool.tile([NT128, 128], F32, tag="ncov")
                    covf_sb = outpool.tile([NT128, 128], F32, tag="covf")
                    nc.sync.dma_start(covf_sb[:], covf[b])
                    nc.vector.tensor_copy(ncov_rows[:], covf_sb[:])
                    nc.sync.dma_start(ncov_out[b], ncov_rows[:])
                    continue

                for blk in range(NBLK):
                    t0 = blk * TBLK
                    th_tiles = []
                    for gc in range(HC):
                        gs = slice(gc * 128, (gc + 1) * 128)
                        f_ps = fpsum.tile([128, TBLK], F32, tag="f")
                        for half in range(TBLK // MMN):
                            hs = slice(half * MMN, (half + 1) * MMN)
                            ts = slice(t0 + half * MMN, t0 + (half + 1) * MMN)
                            nc.tensor.matmul(
                                f_ps[:, hs], weT_sb[:, 0, gs], encT_sb[:, 0, ts],
                                start=True, stop=False,
                            )
                            nc.tensor.matmul(
                                f_ps[:, hs], weT_sb[:, 1, gs], encT_sb[:, 1, ts],
                                start=False, stop=False,
                            )
                            nc.tensor.matmul(
                                f_ps[:, hs], wcdec_sb[:, b, gs], covone[:, ts],
                                start=False, stop=True,
                            )
                        th_sb = thpool.tile([128, TBLK], BF16, tag="th")
                        nc.scalar.activation(th_sb[:], f_ps[:], AF.Tanh)
                        th_tiles.append(th_sb)
                    if stage < 3:
                        p_sb = smpool.tile([128, TBLK], F32, tag="dump")
                        nc.vector.tensor_copy(p_sb[:], th_tiles[0][:])
                        continue
                    # scores for this block: out [t128, 1] per 128-token subtile
                    for sub in range(TBLK // 128):
                        col = blk * (TBLK // 128) + sub
                        ss = slice(sub * 128, (sub + 1) * 128)
                        for gc in range(HC):
                            nc.tensor.matmul(
                                s_ps[:, col : col + 1],
                                th_tiles[gc][:, ss],
                                vt_sb[:, gc : gc + 1],
                                start=(gc == 0), stop=(gc == HC - 1),
                            )

                if stage < 3:
                    continue
                if stage < 4:
                    arows = outpool.tile([128, NT128], F32, tag="arow")
                    nc.scalar.activation(arows[:], s_ps[:], AF.Exp)
                    nc.sync.dma_start(attn_out[b][0:64, 0:64], arows[0:64, 0:64])
                    continue

                # ---- softmax over [128 tokens x 64 tiles] ----
                p_sb = smpool.tile([128, NT128], F32, tag="p")
                nc.scalar.activation(p_sb[:], s_ps[:], AF.Exp)
                pm_sb = smpool.tile([128, NT128], F32, tag="pm")
                rowsum = smpool.tile([128, 1], F32, tag="rowsum")
                nc.vector.tensor_mul(pm_sb[:], p_sb[:], maskT_sb[:])
                if stage == 411:
                    nc.sync.dma_start(attn_out[b][0:32, 0:64], pm_sb[0:32, 0:64])
                    continue
                nc.vector.tensor_reduce(
                    rowsum[:], pm_sb[:], axis=mybir.AxisListType.X, op=OP.add
                )
                if stage == 41:
                    nc.sync.dma_start(attn_out[b][0:32, 0:64], pm_sb[0:32, 0:64])
                    nc.sync.dma_start(attn_out[b][0:1, 0:1], rowsum[0:1, :])
                    continue
                tot_ps = tinypsum.tile([1, 1], F32, tag="tiny")
                nc.tensor.matmul(tot_ps[:], rowsum[:], ones128[:], start=True, stop=True)
                rtot = smpool.tile([1, 1], F32, tag="rtot")
                nc.vector.reciprocal(rtot[:], tot_ps[:])
                if stage == 42:
                    nc.sync.dma_start(attn_out[b][0:1, 0:1], rtot[:])
                    continue
                rb_ps = tinypsum.tile([128, 1], F32, tag="tiny")
                nc.tensor.matmul(rb_ps[:], onesrow[:], rtot[:], start=True, stop=True)
                rb_sb = smpool.tile([128, 1], F32, tag="rbs")
                nc.vector.tensor_copy(rb_sb[:], rb_ps[:])
                if stage == 43:
                    nc.sync.dma_start(attn_out[b][0:64, 0:1], rb_sb[0:64, :])
                    continue
                attn_sb = smpool.tile([128, NT128], F32, tag="attn")
                nc.vector.tensor_scalar_mul(attn_sb[:], pm_sb[:], rb_sb[:])
                attn_bf = smpool.tile([128, NT128], BF16, tag="attnb")
                nc.vector.tensor_copy(attn_bf[:], attn_sb[:])

                if stage < 5:
                    nc.sync.dma_start(attn_out[b][0:32, 0:64], attn_sb[0:32, 0:64])
                    continue

                # ---- attn rows (transpose) + new_cov + outputs ----
                tr_ps = tinypsum.tile([NT128, 128], F32, tag="tiny")
                nc.tensor.transpose(tr_ps[:], attn_sb[:], ident[:])
                attn_rows = outpool.tile([NT128, 128], F32, tag="arow")
                nc.vector.tensor_copy(attn_rows[:], tr_ps[:])
                nc.sync.dma_start(attn_out[b], attn_rows[:])

                covf_sb = outpool.tile([NT128, 128], F32, tag="covf")
                nc.sync.dma_start(covf_sb[:], covf[b])
                ncov_rows = outpool.tile([NT128, 128], F32, tag="ncov")
                nc.vector.tensor_tensor(
                    out=ncov_rows[:], in0=covf_sb[:], in1=attn_rows[:], op=OP.add
                )
                nc.vector.tensor_scalar(
                    out=ncov_rows[:], in0=ncov_rows[:], scalar1=0.0, scalar2=1.0,
                    op0=OP.max, op1=OP.min,
                )
                nc.sync.dma_start(ncov_out[b], ncov_rows[:])

                # ---- context vector: c_t = sum_t attn[t] * enc[t, :] ----
                ct_ps = tinypsum.tile([1, H], F32, tag="tiny")
                for j in range(NT128):
                    nc.tensor.matmul(
                        ct_ps[:], attn_bf[:, j : j + 1], encN_sb[:, j, :],
                        start=(j == 0), stop=(j == NT128 - 1),
                    )
                ct_sb = outpool.tile([1, H], F32, tag="ct")
                nc.vector.tensor_copy(ct_sb[:], ct_ps[:])
                nc.sync.dma_start(ct_out[b : b + 1], ct_sb[:])
            _loop.close()

    return nc


def _get_nc():
    if not _nc_cache:
        nc = _build_program()
        nc.finalize()  # Bacc: runs reg-alloc + sync-wait splitting passes
        _nc_cache.append(nc)
    return _nc_cache[0]


def prepare_in_maps(s_t_hat, encoder_outputs, enc_padding_mask, coverage,
                    attn_dist_node_to_token, We, Wd, bd, wc, v):
    bf = ml_dtypes.bfloat16
    s_t_hat = np.asarray(s_t_hat, np.float32)
    enc = np.ascontiguousarray(np.asarray(encoder_outputs, np.float32))
    mask = np.asarray(enc_padding_mask, np.float32)
    cov = np.asarray(coverage, np.float32)
    We = np.asarray(We, np.float32)
    Wd = np.asarray(Wd, np.float32)
    bd = np.asarray(bd, np.float32)
    wc = np.asarray(wc, np.float32)
    v = np.asarray(v, np.float32)

    # host-side prep (layout/sharding): decoder projection + weight layouts
    dec = s_t_hat @ Wd.T + bd                                   # [B, H]
    weT_np = np.ascontiguousarray(We.T.reshape(HC, 128, H)).astype(bf)
    vt_np = np.ascontiguousarray(v.reshape(H, 1)).astype(bf)
    # enc, both layouts, bf16
    encT_np = np.ascontiguousarray(
        enc.transpose(0, 2, 1).reshape(B, HC, 128, T)
    ).astype(bf)
    encN_np = np.ascontiguousarray(
        enc.reshape(B, NT128, 128, H).transpose(0, 2, 1, 3)
    ).astype(bf)
    covb_np = np.empty((B, 2, T), np.float32)
    covb_np[:, 0, :] = cov
    covb_np[:, 1, :] = 1.0
    covb_np = covb_np.astype(bf)
    covf_np = np.ascontiguousarray(cov.reshape(B, NT128, 128))
    maskT_np = np.ascontiguousarray(
        mask.reshape(B, NT128, 128).transpose(0, 2, 1)
    )
    wcdec_np = np.empty((B, 2, H), np.float32)
    wcdec_np[:, 0, :] = wc[None, :]
    wcdec_np[:, 1, :] = dec
    wcdec_np = wcdec_np.astype(bf)

    in_maps = []
    for c in range(NCORES):
        bs = slice(c * BL, (c + 1) * BL)
        in_maps.append({
            "encT": encT_np[bs],
            "encN": encN_np[bs],
            "weT": weT_np,
            "wcdec": wcdec_np[bs],
            "vt": vt_np,
            "covb": covb_np[bs],
            "covf": covf_np[bs],
            "maskT": maskT_np[bs],
        })
    return in_maps


def assemble_outputs(results):
    attn = np.concatenate(
        [r["attn_out"].reshape(BL, T) for r in results], axis=0
    ).astype(np.float32)
    ncov = np.concatenate(
        [r["ncov_out"].reshape(BL, T) for r in results], axis=0
    ).astype(np.float32)
    ct = np.concatenate(
        [r["ct_out"] for r in results], axis=0
    ).astype(np.float32)
    return ct, attn, ncov


def kernel(**inputs):
    in_maps = prepare_in_maps(**inputs)
    nc = _get_nc()
    res = run_bass_kernel_spmd(nc, in_maps, list(range(NCORES)))
    LAST["res"] = res
    return assemble_outputs(res.results)
